# revision 30
# baseline (speedup 1.0000x reference)
"""Distributed 2-layer GCN for Trainium2 (8 NeuronCores).

Math (matches the reference):
    x   = embed[tok] @ Wn.T + bn
    deg = in-degree over (edges + self loops); dinv = 1/sqrt(deg)
    per layer l (W, b):   h = x @ W.T
                          z[d] = sum_{e: dst=d} dinv[src] dinv[d] h[src]
                          out  = z + b ; relu between layers

Decomposition:  g = dinv * (x @ W.T) per-node row table, so
    z[d] = dinv[d] * sum_{e: dst=d} g[src]  -- per-edge coefficients vanish.
Layer-1 table folds the prep matmul:  g1 = dinv * (embed[tok] @ A1.T),
A1 = W1 @ Wn (biases are all zero for this problem).  Between layers
(flipped layout, z1 kept [feat, node]):  relu(dinv*z1) = dinv*relu(z1), so
    g2 = dinv^2 * (relu(z1).T @ W2.T)   (row scaling commutes).

Sharding: nodes by contiguous blocks of 12500 per core (dst ownership).
Each core computes g1 for its shard (bf16 embed gathered in TRANSPOSE mode
so the prep matmul needs no on-chip transposes), AllGather -> g1 table
[8*PREP_ROWS, 128] bf16; edges bucketed by (dst-window of 128, src-region
of table/4) with per-slot int16 region-relative indices; dma_gather pulls
message rows; a one-hot S matrix (DVE is_equal against an iota row)
scatter-adds 128 messages per TensorE matmul into a per-window PSUM
accumulator.  Layer 1 accumulates transposed ([feat, dst]) so the
inter-layer g2 matmul consumes it directly (no transposes); layer 2
accumulates [dst, feat] so the output DMA is direct.  All idx/col streams
are preloaded into SBUF in bulk; per-window outputs are staged and written
in per-group batched DMAs.
"""
import sys
import numpy as np

sys.path.insert(0, "/opt/trn_rl_repo")

import ml_dtypes
import concourse.bass as bass
import concourse.bacc as bacc
import concourse.mybir as mybir
import concourse.tile as tile
from concourse.bass_utils import run_bass_kernel_spmd

BF = ml_dtypes.bfloat16

# ---------------- configuration ----------------

class Cfg:
    def __init__(self, N, E, V, DIN, D, NC=8, WB1=4, CALLBLK=8, SGW=7, NREG=4, NQ=4):
        self.N, self.E, self.V, self.DIN, self.D = N, E, V, DIN, D
        self.NC = NC
        self.NPC = N // NC
        self.NW = -(-self.NPC // 128)          # dst windows per core
        self.DST_ROWS = self.NW * 128
        self.HALF = min(((V // 2 + 127) // 128) * 128, 32767 - 128)  # vocab split
        assert self.HALF <= 32767 and V - self.HALF <= 32767
        self.WB1 = WB1                         # prep tiles per embed-gather call
        self.PREP_CALL = WB1 * 128             # idx per embed-gather call
        self.CALLBLK = CALLBLK                 # max 128-slot blocks per msg gather call
        self.SGW = SGW                         # windows per supergroup (PSUM banks)
        assert self.NW % SGW == 0
        self.NREG = NREG
        self.NQ = NQ                           # SWDGE queues


FULL = Cfg(N=100000, E=1600000, V=50000, DIN=256, D=128)


def _wrap_idx16(idx_linear):
    """dma_gather index layout: slot i -> [i%16, i//16]; [128, n/16] int16."""
    n = idx_linear.shape[0]
    assert n % 16 == 0
    arr = idx_linear.astype(np.int16).reshape(n // 16, 16).T
    return np.ascontiguousarray(np.tile(arr, (8, 1)))


# ---------------- host preprocessing ----------------

class Prep:
    pass


def preprocess(cfg, node_tokens, edge_index):
    c = cfg
    tok = np.asarray(node_tokens).astype(np.int64).ravel()
    ei = np.asarray(edge_index).astype(np.int64)
    src, dst = ei[0], ei[1]

    deg = np.bincount(dst, minlength=c.N).astype(np.float64) + 1.0
    dinv = (1.0 / np.sqrt(deg)).astype(np.float32)

    # --- per-core node orderings
    pos_p = np.empty(c.N, np.int64)   # prep slot (gather order, vocab-half grouped)
    pos_d = np.empty(c.N, np.int64)   # dst rank (degree-sorted windows)
    realA = np.empty(c.NC, np.int64)
    per_core = []
    for k in range(c.NC):
        g0 = k * c.NPC
        nodes = np.arange(g0, g0 + c.NPC)
        t = tok[nodes]
        isB = t >= c.HALF
        nA = int((~isB).sum())
        realA[k] = nA
        ordp = np.argsort(isB, kind="stable")
        per_core.append((nodes, t, isB, ordp, nA))

    PC = c.PREP_CALL
    NA = int(-(-(realA.max()) // PC) * PC)
    NBmax = int((c.NPC - realA).max())
    NB_ROWS = int(-(-NBmax // PC) * PC)
    PREP_ROWS = NA + NB_ROWS
    # q1 must be a whole number of prep calls
    PREP_ROWS = int(-(-PREP_ROWS // (c.NREG * PC)) * (c.NREG * PC))
    q1 = PREP_ROWS // c.NREG
    q2 = c.DST_ROWS // c.NREG
    REG1 = c.NC * q1
    REG2 = c.NC * q2
    assert REG1 <= 32767 and REG2 <= 32767, (REG1, REG2)
    PREP_TILES = PREP_ROWS // 128
    NCALL1 = PREP_ROWS // PC

    p = Prep()
    p.cfg = c
    p.NA, p.PREP_ROWS, p.REG1, p.REG2 = NA, PREP_ROWS, REG1, REG2
    p.PREP_TILES, p.NCALL1 = PREP_TILES, NCALL1

    prep_idx = np.zeros((c.NC, PREP_ROWS), np.int64)
    dinv_p = np.zeros((c.NC, PREP_ROWS), np.float32)
    dinv_d = np.zeros((c.NC, c.DST_ROWS), np.float32)
    order_d = np.zeros((c.NC, c.NPC), np.int64)
    for k in range(c.NC):
        nodes, t, isB, ordp, nA = per_core[k]
        slots = np.empty(c.NPC, np.int64)
        slots[ordp[:nA]] = np.arange(nA)
        slots[ordp[nA:]] = NA + np.arange(c.NPC - nA)
        pos_p[nodes] = slots
        prep_idx[k][slots] = np.where(isB, t - c.HALF, t)
        dinv_p[k][slots] = dinv[nodes]
        od = np.argsort(-deg[nodes], kind="stable")
        rank = np.empty(c.NPC, np.int64)
        rank[od] = np.arange(c.NPC)
        pos_d[nodes] = rank
        order_d[k] = od
        dinv_d[k][rank] = dinv[nodes]

    # DRAM row layouts follow the batched stage writes (p-major interleave):
    # g1b row for prep slot s, g2b/out row for dst rank r.
    def lin1(s):
        t, pp = s // 128, s % 128
        return (t // c.WB1) * (128 * c.WB1) + pp * c.WB1 + (t % c.WB1)

    def lin2(r):
        w, pp = r // 128, r % 128
        return (w // c.SGW) * (128 * c.SGW) + pp * c.SGW + (w % c.SGW)

    core_of = np.arange(c.N) // c.NPC
    l1 = lin1(pos_p)
    l2 = lin2(pos_d)
    y0reg = l1 // q1
    y0idx = core_of * q1 + (l1 % q1)   # region-relative table row
    y1reg = l2 // q2
    y1idx = core_of * q2 + (l2 % q2)

    # --- edges incl self loops
    es = np.concatenate([src, np.arange(c.N)])
    ed = np.concatenate([dst, np.arange(c.N)])
    ecore = ed // c.NPC
    e_dl = pos_d[ed]                 # local dst rank (within owner core)
    e_w = e_dl // 128
    e_col = (e_dl % 128).astype(np.float32)

    def layer_streams(e_r, e_i):
        """Per-core flat idx/col arrays + global (shared) packed schedule.

        Buckets (dst-window, src-region) are packed back-to-back in the slot
        stream (no per-bucket rounding to 128); a 128-slot block at a window
        boundary is processed by one matmul instance per window it overlaps,
        each with its own col column (other windows' slots masked to -1)."""
        cnt = np.zeros((c.NC, c.NW, c.NREG), np.int64)
        per_core_order = []
        for k in range(c.NC):
            m = ecore == k
            wk, rk, ik, colk = e_w[m], e_r[m], e_i[m], e_col[m]
            o = np.lexsort((colk, rk, wk))
            wk, rk, ik, colk = wk[o], rk[o], ik[o], colk[o]
            cnt[k] = np.bincount(wk * c.NREG + rk, minlength=c.NW * c.NREG).reshape(c.NW, c.NREG)
            per_core_order.append((wk, rk, ik, colk))
        cmax = cnt.max(axis=0)                # [NW, NREG] shared (max-over-core) counts
        groups = [list(range(s, s + c.SGW)) for s in range(0, c.NW, c.SGW)]
        bucket_off = np.zeros((c.NW, c.NREG), np.int64)
        calls = []          # (grp_idx, region, b0, nbc, live_idx)
        call_insts = []     # per call: list of (iid, blk, w, kl)
        insts = []          # (blk, w) in processing order
        inst_id = {}
        first_inst, last_inst = {}, {}
        cur = 0             # global slot cursor; 128-aligned at run starts
        for gi, grp in enumerate(groups):
            for rr in range(c.NREG):
                r = (gi + rr) % c.NREG
                run_s0 = cur
                for w in grp:
                    bucket_off[w, r] = cur
                    cur += int(cmax[w, r])
                run_live = cur - run_s0
                if run_live == 0:
                    continue
                run_insts = []
                for w in grp:
                    s0, s1 = bucket_off[w, r], bucket_off[w, r] + int(cmax[w, r])
                    if s1 == s0:
                        continue
                    for blk in range(s0 // 128, (s1 - 1) // 128 + 1):
                        iid = len(insts)
                        inst_id[(blk, w)] = iid
                        insts.append((blk, w))
                        run_insts.append((iid, blk, w))
                        if w not in first_inst:
                            first_inst[w] = iid
                        last_inst[w] = iid
                cur = -(-cur // 128) * 128
                run_b0, run_nblk = run_s0 // 128, (cur - run_s0) // 128
                ri = 0
                b0 = run_b0
                while b0 < run_b0 + run_nblk:
                    nbc = min(c.CALLBLK, run_b0 + run_nblk - b0)
                    live = min(run_s0 + run_live - b0 * 128, nbc * 128)
                    live = min(max(-(-live // 16) * 16, 16), nbc * 128)
                    ci = []
                    while ri < len(run_insts) and run_insts[ri][1] < b0 + nbc:
                        iid, blk, w = run_insts[ri]
                        kl = min(128, max(16, live - (blk - b0) * 128))
                        ci.append((iid, blk, w, kl))
                        ri += 1
                    calls.append((gi, r, b0, nbc, live))
                    call_insts.append(ci)
                    b0 += nbc
        TOTBLK = cur // 128
        NINST = len(insts)
        idxs, cols = [], []
        for k in range(c.NC):
            wk, rk, ik, colk = per_core_order[k]
            gid = wk * c.NREG + rk
            starts = np.zeros(c.NW * c.NREG, np.int64)
            cc = np.bincount(gid, minlength=c.NW * c.NREG)
            starts[1:] = np.cumsum(cc)[:-1]
            posin = np.arange(len(gid)) - starts[gid]
            slot = bucket_off.reshape(-1)[gid] + posin
            idx_flat = np.zeros(TOTBLK * 128, np.int64)
            idx_flat[slot] = ik
            for (_gi, _r, _b0, _nbc, _nreg) in calls:
                if _nreg < _nbc * 128:
                    idx_flat[_b0 * 128 + _nreg:(_b0 + _nbc) * 128] = -1
            col_flat = np.full((NINST, 128), -1.0, np.float32)
            iid_of = np.array([inst_id[(s // 128, w)] for s, w in zip(slot, wk)],
                              np.int64)
            col_flat[iid_of, slot % 128] = colk
            idxs.append(_wrap_idx16(idx_flat))
            cols.append(np.ascontiguousarray(col_flat.T))
        sched = dict(groups=groups, calls=calls, call_insts=call_insts,
                     first_inst=first_inst, last_inst=last_inst,
                     TOTBLK=TOTBLK, NINST=NINST)
        return sched, idxs, cols

    p.s1, p.idx1, p.col1 = layer_streams(y0reg[es], y0idx[es])
    p.s2, p.idx2, p.col2 = layer_streams(y1reg[es], y1idx[es])
    p.prep_idx = [_wrap_idx16(prep_idx[k]) for k in range(c.NC)]
    p.dinv_p = [np.ascontiguousarray(dinv_p[k].reshape(PREP_TILES, 128).T) for k in range(c.NC)]
    p.dinv_d = [np.ascontiguousarray(dinv_d[k].reshape(c.NW, 128).T) for k in range(c.NC)]
    p.order_d = order_d
    p.lin2_npc = lin2(np.arange(c.NPC))
    return p


# ---------------- device kernel ----------------

def build_nc(p, sim_collectives=False, shared_tables=False, msg_bufs=3, mp_zpool=False):
    c = p.cfg
    f32, bf16, i16 = mybir.dt.float32, mybir.dt.bfloat16, mybir.dt.int16
    D, DIN = c.D, c.DIN
    KIN = DIN // 128
    PC = c.PREP_CALL
    T1, T2 = p.s1["TOTBLK"], p.s2["TOTBLK"]
    N1, N2 = p.s1["NINST"], p.s2["NINST"]
    nc = bacc.Bacc("TRN2", target_bir_lowering=False, debug=False,
                   num_devices=c.NC, num_swdge_queues=c.NQ)

    embed = nc.dram_tensor("embed", [c.V, DIN], bf16, kind="ExternalInput").ap()
    prep_idx_d = nc.dram_tensor("prep_idx", [128, p.PREP_ROWS // 16], i16, kind="ExternalInput").ap()
    idx1_d = nc.dram_tensor("idx1", [128, T1 * 8], i16, kind="ExternalInput").ap()
    col1_d = nc.dram_tensor("col1", [128, N1], f32, kind="ExternalInput").ap()
    idx2_d = nc.dram_tensor("idx2", [128, T2 * 8], i16, kind="ExternalInput").ap()
    col2_d = nc.dram_tensor("col2", [128, N2], f32, kind="ExternalInput").ap()
    dinvp_d = nc.dram_tensor("dinvp", [128, p.PREP_TILES], f32, kind="ExternalInput").ap()
    dinvd_d = nc.dram_tensor("dinvd", [128, c.NW], f32, kind="ExternalInput").ap()
    a1t_d = nc.dram_tensor("a1t", [DIN, D], bf16, kind="ExternalInput").ap()
    w2t_d = nc.dram_tensor("w2t", [D, D], bf16, kind="ExternalInput").ap()
    iota_d = nc.dram_tensor("iota", [128, D], bf16, kind="ExternalInput").ap()
    outp = nc.dram_tensor("out", [c.DST_ROWS, D], f32, kind="ExternalOutput").ap()

    q1 = p.PREP_ROWS // c.NREG
    q2 = c.DST_ROWS // c.NREG
    CPP = p.NCALL1 // c.NREG          # prep calls per AllGather piece

    with tile.TileContext(nc) as tc:
        with (
            tc.tile_pool(name="dram", bufs=1, space="DRAM") as dpool,
            tc.tile_pool(name="const", bufs=1) as cpool,
            tc.tile_pool(name="msg", bufs=3) as msgpool,
            tc.tile_pool(name="emb", bufs=2) as embpool,
            tc.tile_pool(name="s", bufs=8) as spool,
            tc.tile_pool(name="stg", bufs=2) as stgpool,
            tc.tile_pool(name="post", bufs=4) as postpool,
            tc.tile_pool(name="zp", bufs=c.SGW, space="PSUM") as zpool,
            tc.tile_pool(name="aux", bufs=1, space="PSUM") as auxpool,
        ):
            f_space = "Shared" if (shared_tables and not sim_collectives) else "Local"
            g1b = dpool.tile([p.PREP_ROWS, D], bf16)
            g1fs = [dpool.tile([p.REG1, D], bf16, addr_space=f_space,
                               name=f"g1f_r{r}") for r in range(c.NREG)]
            g2b = dpool.tile([c.DST_ROWS, D], bf16)
            g2fs = [dpool.tile([p.REG2, D], bf16, addr_space=f_space,
                               name=f"g2f_r{r}") for r in range(c.NREG)]

            iota_t = cpool.tile([128, D], bf16)
            nc.sync.dma_start(iota_t[:], iota_d[:])
            a1t_t = cpool.tile([128, KIN, D], bf16)
            for kk in range(KIN):
                nc.sync.dma_start(a1t_t[:, kk, :], a1t_d[kk * 128:(kk + 1) * 128, :])
            w2t_t = cpool.tile([128, D], bf16)
            nc.sync.dma_start(w2t_t[:], w2t_d[:])
            dinvp_t = cpool.tile([128, p.PREP_TILES], f32)
            nc.sync.dma_start(dinvp_t[:], dinvp_d[:])
            dinvd_t = cpool.tile([128, c.NW], f32)
            nc.sync.dma_start(dinvd_t[:], dinvd_d[:])
            dinvd2_t = cpool.tile([128, c.NW], f32)
            nc.vector.tensor_mul(dinvd2_t[:], dinvd_t[:], dinvd_t[:])
            pidx_t = cpool.tile([128, p.PREP_ROWS // 16], i16)
            nc.sync.dma_start(pidx_t[:], prep_idx_d[:])
            # bulk idx/col streams for both layers
            idx1_t = cpool.tile([128, T1 * 8], i16)
            nc.sync.dma_start(idx1_t[:], idx1_d[:])
            col1_t = cpool.tile([128, N1], f32)
            nc.sync.dma_start(col1_t[:], col1_d[:])
            idx2_t = cpool.tile([128, T2 * 8], i16)
            nc.sync.dma_start(idx2_t[:], idx2_d[:])
            col2_t = cpool.tile([128, N2], f32)
            nc.sync.dma_start(col2_t[:], col2_d[:])

            mcall = 0
            act_copy = mybir.ActivationFunctionType.Copy
            act_relu = mybir.ActivationFunctionType.Relu

            # ---------------- prep: g1 = dinv * (embed[tok] @ A1.T) ----------------
            halfA = embed[0:c.HALF, :]
            halfB = embed[c.HALF:c.V, :]
            n_callsA = p.NA // PC

            def emit_ag1(q):
                if sim_collectives:
                    for kk in range(c.NC):
                        nc.sync.dma_start(g1fs[q][kk * q1:(kk + 1) * q1, :],
                                          g1b[q * q1:(q + 1) * q1, :])
                    return
                nc.gpsimd.collective_compute(
                    "AllGather", mybir.AluOpType.bypass,
                    ins=[g1b[q * q1:(q + 1) * q1, :]],
                    outs=[g1fs[q][:]],
                    replica_groups=[list(range(c.NC))],
                )

            ag1_next = 0
            for call in range(p.NCALL1):
                while ag1_next < c.NREG and (ag1_next + 1) * CPP <= call:
                    emit_ag1(ag1_next)
                    ag1_next += 1
                srcap = halfA if call < n_callsA else halfB
                # transpose-mode gather: [din-part, KIN, tokens] bf16
                et = embpool.tile([128, KIN, PC], bf16, tag="emb")
                nc.gpsimd.dma_gather(
                    et[:], srcap, pidx_t[:, call * (PC // 16):(call + 1) * (PC // 16)],
                    num_idxs=PC, num_idxs_reg=PC, elem_size=DIN, transpose=True,
                    queue_num=0,
                )
                g1s = stgpool.tile([128, c.WB1, D], bf16, tag="g1s")
                for j in range(c.WB1):
                    t_idx = call * c.WB1 + j
                    if mp_zpool:
                        mpt = zpool.tile([128, D], f32, tag="z", name=f"mp_{t_idx}")
                    else:
                        mpt = auxpool.tile([128, D], f32, tag="aux", name=f"mp_{t_idx}")
                    for kk in range(KIN):
                        nc.tensor.matmul(mpt[:], et[:, kk, j * 128:(j + 1) * 128],
                                         a1t_t[:, kk, :],
                                         start=(kk == 0), stop=(kk == KIN - 1))
                    nc.scalar.activation(g1s[:, j, :], mpt[:], act_copy,
                                         scale=dinvp_t[:, t_idx:t_idx + 1])
                nc.sync.dma_start(g1b[call * PC:(call + 1) * PC, :], g1s[:])

            while ag1_next < c.NREG:
                emit_ag1(ag1_next)
                ag1_next += 1

            # ---------------- layers ----------------
            def layer(sched, idx_t, col_t, tables, is_last, grp_hook=None):
                groups, calls = sched["groups"], sched["calls"]
                call_insts = sched["call_insts"]
                first_inst, last_inst = sched["first_inst"], sched["last_inst"]
                nonlocal mcall
                call_i = 0
                ncalls = len(calls)
                zt = {}
                li = 2 if is_last else 1
                for gi, grp in enumerate(groups):
                    if grp_hook is not None:
                        grp_hook(gi)
                    for w in grp:
                        zt[w] = zpool.tile([128, D], f32, tag="z", name=f"zt{li}_w{w}")
                    while call_i < ncalls and calls[call_i][0] == gi:
                        _, r, b0, nbc, nreg = calls[call_i]
                        ci = call_insts[call_i]
                        call_i += 1
                        msg_t = msgpool.tile([128, c.CALLBLK, D], bf16, tag="msg")
                        nc.gpsimd.dma_gather(
                            msg_t[:, 0:nbc, :], tables[r][:],
                            idx_t[:, b0 * 8:(b0 + nbc) * 8],
                            num_idxs=nbc * 128, num_idxs_reg=nreg, elem_size=D,
                            queue_num=1 + mcall % 3,
                        )
                        mcall += 1
                        for iid, blk, w, kl in ci:
                            b = blk - b0
                            s_t = spool.tile([128, D], bf16, tag="s")
                            nc.vector.tensor_scalar(
                                out=s_t[:], in0=iota_t[:], scalar1=col_t[:, iid:iid + 1],
                                scalar2=None, op0=mybir.AluOpType.is_equal,
                            )
                            if is_last:   # zt[dst, feat] += S.T @ msg
                                nc.tensor.matmul(zt[w][:], s_t[0:kl, :], msg_t[0:kl, b, :],
                                                 start=(iid == first_inst[w]),
                                                 stop=(iid == last_inst[w]))
                            else:         # zt[feat, dst] += msg.T @ S
                                nc.tensor.matmul(zt[w][:], msg_t[0:kl, b, :], s_t[0:kl, :],
                                                 start=(iid == first_inst[w]),
                                                 stop=(iid == last_inst[w]))
                    # post-process this group's windows into a staged batch write
                    stage = stgpool.tile([128, c.SGW, D], f32 if is_last else bf16,
                                         tag="outs" if is_last else "g2s")
                    for jw, w in enumerate(grp):
                        if not is_last:
                            r1 = postpool.tile([128, D], bf16, tag="r1")
                            nc.scalar.activation(r1[:], zt[w][:], act_relu)
                            gp = auxpool.tile([128, D], f32, tag="aux")
                            nc.tensor.matmul(gp[:], r1[:], w2t_t[:], start=True, stop=True)
                            nc.scalar.activation(stage[:, jw, :], gp[:], act_copy,
                                                 scale=dinvd2_t[:, w:w + 1])
                        else:
                            nc.scalar.activation(stage[:, jw, :], zt[w][:], act_copy,
                                                 scale=dinvd_t[:, w:w + 1])
                        del zt[w]
                    dst = outp if is_last else g2b
                    nc.sync.dma_start(
                        dst[gi * c.SGW * 128:(gi + 1) * c.SGW * 128, :], stage[:])

            def emit_ag2(q):
                if sim_collectives:
                    for kk in range(c.NC):
                        nc.sync.dma_start(g2fs[q][kk * q2:(kk + 1) * q2, :],
                                          g2b[q * q2:(q + 1) * q2, :])
                    return
                nc.gpsimd.collective_compute(
                    "AllGather", mybir.AluOpType.bypass,
                    ins=[g2b[q * q2:(q + 1) * q2, :]],
                    outs=[g2fs[q][:]],
                    replica_groups=[list(range(c.NC))],
                )

            ag2_state = [0]

            def ag2_hook(gi):
                while (ag2_state[0] < c.NREG and
                       (ag2_state[0] + 1) * q2 <= gi * c.SGW * 128):
                    emit_ag2(ag2_state[0])
                    ag2_state[0] += 1

            layer(p.s1, idx1_t, col1_t, g1fs, is_last=False, grp_hook=ag2_hook)
            while ag2_state[0] < c.NREG:
                emit_ag2(ag2_state[0])
                ag2_state[0] += 1
            layer(p.s2, idx2_t, col2_t, g2fs, is_last=True)
    nc.finalize()
    return nc


# ---------------- host-side weight prep + in_maps ----------------

def make_in_maps(p, embed_table, W_node_w, W_node_b, conv1_w, conv1_b, conv2_w, conv2_b):
    c = p.cfg
    assert np.abs(W_node_b).max() == 0 and np.abs(conv1_b).max() == 0 and np.abs(conv2_b).max() == 0, \
        "nonzero biases not supported by this build (all-zero in this problem)"
    A1 = (np.asarray(conv1_w, np.float64) @ np.asarray(W_node_w, np.float64)).astype(np.float32)
    a1t = np.ascontiguousarray(A1.T).astype(BF)                  # [DIN, D]
    w2t = np.ascontiguousarray(np.asarray(conv2_w, np.float32).T).astype(BF)
    iota = np.tile(np.arange(c.D, dtype=np.float32), (128, 1)).astype(BF)
    emb = np.ascontiguousarray(np.asarray(embed_table, np.float32).astype(BF))
    maps = []
    for k in range(c.NC):
        maps.append({
            "embed": emb,
            "prep_idx": p.prep_idx[k],
            "idx1": p.idx1[k], "col1": p.col1[k],
            "idx2": p.idx2[k], "col2": p.col2[k],
            "dinvp": p.dinv_p[k], "dinvd": p.dinv_d[k],
            "a1t": a1t, "w2t": w2t, "iota": iota,
        })
    return maps


def assemble(p, results):
    c = p.cfg
    out = np.empty((c.N, c.D), np.float32)
    for k in range(c.NC):
        r = results[k]["out"]
        out[k * c.NPC + p.order_d[k]] = r[p.lin2_npc]
    return out


_CACHE = {}

def kernel(node_tokens, edge_index, embed_table, W_node_w, W_node_b,
           conv1_w, conv1_b, conv2_w, conv2_b):
    cfg = FULL
    p = preprocess(cfg, node_tokens, edge_index)
    key = (p.PREP_ROWS, p.s1["TOTBLK"], p.s2["TOTBLK"],
           tuple(p.s1["calls"]), tuple(p.s2["calls"]))
    if key not in _CACHE:
        _CACHE[key] = build_nc(p)
    nc = _CACHE[key]
    maps = make_in_maps(p, embed_table, W_node_w, W_node_b, conv1_w, conv1_b, conv2_w, conv2_b)
    res = run_bass_kernel_spmd(nc, maps, core_ids=list(range(cfg.NC)))
    return assemble(p, res.results)


# revision 31
# speedup vs baseline: 1.0933x; 1.0933x over previous
"""Distributed 2-layer GCN for Trainium2 (8 NeuronCores).

Math (matches the reference):
    x   = embed[tok] @ Wn.T + bn
    deg = in-degree over (edges + self loops); dinv = 1/sqrt(deg)
    per layer l (W, b):   h = x @ W.T
                          z[d] = sum_{e: dst=d} dinv[src] dinv[d] h[src]
                          out  = z + b ; relu between layers

Decomposition:  g = dinv * (x @ W.T) per-node row table, so
    z[d] = dinv[d] * sum_{e: dst=d} g[src]  -- per-edge coefficients vanish.
Layer-1 table folds the prep matmul:  g1 = dinv * (embed[tok] @ A1.T),
A1 = W1 @ Wn (biases are all zero for this problem).  Between layers
(flipped layout, z1 kept [feat, node]):  relu(dinv*z1) = dinv*relu(z1), so
    g2 = dinv^2 * (relu(z1).T @ W2.T)   (row scaling commutes).

Sharding: nodes by contiguous blocks of 12500 per core (dst ownership).
Each core computes g1 for its shard (bf16 embed gathered in TRANSPOSE mode
so the prep matmul needs no on-chip transposes), AllGather -> g1 table
[8*PREP_ROWS, 128] bf16; edges bucketed by (dst-window of 128, src-region
of table/4) with per-slot int16 region-relative indices; dma_gather pulls
message rows; a one-hot S matrix (DVE is_equal against an iota row)
scatter-adds 128 messages per TensorE matmul into a per-window PSUM
accumulator.  Layer 1 accumulates transposed ([feat, dst]) so the
inter-layer g2 matmul consumes it directly (no transposes); layer 2
accumulates [dst, feat] so the output DMA is direct.  All idx/col streams
are preloaded into SBUF in bulk; per-window outputs are staged and written
in per-group batched DMAs.
"""
import sys
import numpy as np

sys.path.insert(0, "/opt/trn_rl_repo")

import ml_dtypes
import concourse.bass as bass
import concourse.bacc as bacc
import concourse.mybir as mybir
import concourse.tile as tile
from concourse.bass_utils import run_bass_kernel_spmd

BF = ml_dtypes.bfloat16

# ---------------- configuration ----------------

class Cfg:
    def __init__(self, N, E, V, DIN, D, NC=8, WB1=4, CALLBLK=8, SGW=7, NREG=4, NQ=4):
        self.N, self.E, self.V, self.DIN, self.D = N, E, V, DIN, D
        self.NC = NC
        self.NPC = N // NC
        self.NW = -(-self.NPC // 128)          # dst windows per core
        self.DST_ROWS = self.NW * 128
        self.HALF = min(((V // 2 + 127) // 128) * 128, 32767 - 128)  # vocab split
        assert self.HALF <= 32767 and V - self.HALF <= 32767
        self.WB1 = WB1                         # prep tiles per embed-gather call
        self.PREP_CALL = WB1 * 128             # idx per embed-gather call
        self.CALLBLK = CALLBLK                 # max 128-slot blocks per msg gather call
        self.SGW = SGW                         # windows per supergroup (PSUM banks)
        assert self.NW % SGW == 0
        self.NREG = NREG
        self.NQ = NQ                           # SWDGE queues


FULL = Cfg(N=100000, E=1600000, V=50000, DIN=256, D=128)


def _wrap_idx16(idx_linear):
    """dma_gather index layout: slot i -> [i%16, i//16]; [128, n/16] int16."""
    n = idx_linear.shape[0]
    assert n % 16 == 0
    arr = idx_linear.astype(np.int16).reshape(n // 16, 16).T
    return np.ascontiguousarray(np.tile(arr, (8, 1)))


# ---------------- host preprocessing ----------------

class Prep:
    pass


def preprocess(cfg, node_tokens, edge_index):
    c = cfg
    tok = np.asarray(node_tokens).astype(np.int64).ravel()
    ei = np.asarray(edge_index).astype(np.int64)
    src, dst = ei[0], ei[1]

    deg = np.bincount(dst, minlength=c.N).astype(np.float64) + 1.0
    dinv = (1.0 / np.sqrt(deg)).astype(np.float32)

    # --- per-core node orderings
    pos_p = np.empty(c.N, np.int64)   # prep slot (gather order, vocab-half grouped)
    pos_d = np.empty(c.N, np.int64)   # dst rank (degree-sorted windows)
    realA = np.empty(c.NC, np.int64)
    per_core = []
    for k in range(c.NC):
        g0 = k * c.NPC
        nodes = np.arange(g0, g0 + c.NPC)
        t = tok[nodes]
        isB = t >= c.HALF
        nA = int((~isB).sum())
        realA[k] = nA
        ordp = np.argsort(isB, kind="stable")
        per_core.append((nodes, t, isB, ordp, nA))

    PC = c.PREP_CALL
    NA = int(-(-(realA.max()) // PC) * PC)
    NBmax = int((c.NPC - realA).max())
    NB_ROWS = int(-(-NBmax // PC) * PC)
    PREP_ROWS = NA + NB_ROWS
    # q1 must be a whole number of prep calls
    PREP_ROWS = int(-(-PREP_ROWS // (c.NREG * PC)) * (c.NREG * PC))
    q1 = PREP_ROWS // c.NREG
    q2 = c.DST_ROWS // c.NREG
    REG1 = c.NC * q1
    REG2 = c.NC * q2
    assert REG1 <= 32767 and REG2 <= 32767, (REG1, REG2)
    PREP_TILES = PREP_ROWS // 128
    NCALL1 = PREP_ROWS // PC

    p = Prep()
    p.cfg = c
    p.NA, p.PREP_ROWS, p.REG1, p.REG2 = NA, PREP_ROWS, REG1, REG2
    p.PREP_TILES, p.NCALL1 = PREP_TILES, NCALL1

    prep_idx = np.zeros((c.NC, PREP_ROWS), np.int64)
    dinv_p = np.zeros((c.NC, PREP_ROWS), np.float32)
    dinv_d = np.zeros((c.NC, c.DST_ROWS), np.float32)
    order_d = np.zeros((c.NC, c.NPC), np.int64)
    for k in range(c.NC):
        nodes, t, isB, ordp, nA = per_core[k]
        slots = np.empty(c.NPC, np.int64)
        slots[ordp[:nA]] = np.arange(nA)
        slots[ordp[nA:]] = NA + np.arange(c.NPC - nA)
        pos_p[nodes] = slots
        prep_idx[k][slots] = np.where(isB, t - c.HALF, t)
        dinv_p[k][slots] = dinv[nodes]
        od = np.argsort(-deg[nodes], kind="stable")
        rank = np.empty(c.NPC, np.int64)
        rank[od] = np.arange(c.NPC)
        pos_d[nodes] = rank
        order_d[k] = od
        dinv_d[k][rank] = dinv[nodes]

    # DRAM row layouts follow the batched stage writes (p-major interleave):
    # g1b row for prep slot s, g2b/out row for dst rank r.
    def lin1(s):
        t, pp = s // 128, s % 128
        return (t // c.WB1) * (128 * c.WB1) + pp * c.WB1 + (t % c.WB1)

    def lin2(r):
        w, pp = r // 128, r % 128
        return (w // c.SGW) * (128 * c.SGW) + pp * c.SGW + (w % c.SGW)

    core_of = np.arange(c.N) // c.NPC
    l1 = lin1(pos_p)
    l2 = lin2(pos_d)
    y0reg = l1 // q1
    y0idx = core_of * q1 + (l1 % q1)   # region-relative table row
    y1reg = l2 // q2
    y1idx = core_of * q2 + (l2 % q2)

    # --- edges incl self loops
    es = np.concatenate([src, np.arange(c.N)])
    ed = np.concatenate([dst, np.arange(c.N)])
    ecore = ed // c.NPC
    e_dl = pos_d[ed]                 # local dst rank (within owner core)
    e_w = e_dl // 128
    e_col = (e_dl % 128).astype(np.float32)

    def layer_streams(e_r, e_i):
        """Per-core flat idx/col arrays + global (shared) packed schedule.

        Buckets (dst-window, src-region) are packed back-to-back in the slot
        stream (no per-bucket rounding to 128); a 128-slot block at a window
        boundary is processed by one matmul instance per window it overlaps,
        each with its own col column (other windows' slots masked to -1)."""
        cnt = np.zeros((c.NC, c.NW, c.NREG), np.int64)
        per_core_order = []
        for k in range(c.NC):
            m = ecore == k
            wk, rk, ik, colk = e_w[m], e_r[m], e_i[m], e_col[m]
            o = np.lexsort((colk, rk, wk))
            wk, rk, ik, colk = wk[o], rk[o], ik[o], colk[o]
            cnt[k] = np.bincount(wk * c.NREG + rk, minlength=c.NW * c.NREG).reshape(c.NW, c.NREG)
            per_core_order.append((wk, rk, ik, colk))
        cmax = cnt.max(axis=0)                # [NW, NREG] shared (max-over-core) counts
        groups = [list(range(s, s + c.SGW)) for s in range(0, c.NW, c.SGW)]
        bucket_off = np.zeros((c.NW, c.NREG), np.int64)
        calls = []          # (grp_idx, region, b0, nbc, live_idx)
        call_insts = []     # per call: list of (iid, blk, w, kl)
        insts = []          # (blk, w) in processing order
        inst_id = {}
        first_inst, last_inst = {}, {}
        cur = 0             # global slot cursor; 128-aligned at run starts
        for gi, grp in enumerate(groups):
            for rr in range(c.NREG):
                r = (gi + rr) % c.NREG
                run_s0 = cur
                for w in grp:
                    bucket_off[w, r] = cur
                    cur += int(cmax[w, r])
                run_live = cur - run_s0
                if run_live == 0:
                    continue
                run_insts = []
                for w in grp:
                    s0, s1 = bucket_off[w, r], bucket_off[w, r] + int(cmax[w, r])
                    if s1 == s0:
                        continue
                    for blk in range(s0 // 128, (s1 - 1) // 128 + 1):
                        iid = len(insts)
                        inst_id[(blk, w)] = iid
                        insts.append((blk, w))
                        run_insts.append((iid, blk, w))
                        if w not in first_inst:
                            first_inst[w] = iid
                        last_inst[w] = iid
                cur = -(-cur // 128) * 128
                run_b0, run_nblk = run_s0 // 128, (cur - run_s0) // 128
                ri = 0
                b0 = run_b0
                while b0 < run_b0 + run_nblk:
                    nbc = min(c.CALLBLK, run_b0 + run_nblk - b0)
                    live = min(run_s0 + run_live - b0 * 128, nbc * 128)
                    live = min(max(-(-live // 16) * 16, 16), nbc * 128)
                    ci = []
                    while ri < len(run_insts) and run_insts[ri][1] < b0 + nbc:
                        iid, blk, w = run_insts[ri]
                        kl = min(128, max(16, live - (blk - b0) * 128))
                        ci.append((iid, blk, w, kl))
                        ri += 1
                    calls.append((gi, r, b0, nbc, live))
                    call_insts.append(ci)
                    b0 += nbc
        TOTBLK = cur // 128
        NINST = len(insts)
        idxs, cols = [], []
        for k in range(c.NC):
            wk, rk, ik, colk = per_core_order[k]
            gid = wk * c.NREG + rk
            starts = np.zeros(c.NW * c.NREG, np.int64)
            cc = np.bincount(gid, minlength=c.NW * c.NREG)
            starts[1:] = np.cumsum(cc)[:-1]
            posin = np.arange(len(gid)) - starts[gid]
            slot = bucket_off.reshape(-1)[gid] + posin
            idx_flat = np.zeros(TOTBLK * 128, np.int64)
            idx_flat[slot] = ik
            for (_gi, _r, _b0, _nbc, _nreg) in calls:
                if _nreg < _nbc * 128:
                    idx_flat[_b0 * 128 + _nreg:(_b0 + _nbc) * 128] = -1
            col_flat = np.full((NINST, 128), -1.0, np.float32)
            iid_of = np.array([inst_id[(s // 128, w)] for s, w in zip(slot, wk)],
                              np.int64)
            col_flat[iid_of, slot % 128] = colk
            idxs.append(_wrap_idx16(idx_flat))
            cols.append(np.ascontiguousarray(col_flat.T))
        sched = dict(groups=groups, calls=calls, call_insts=call_insts,
                     first_inst=first_inst, last_inst=last_inst,
                     TOTBLK=TOTBLK, NINST=NINST)
        return sched, idxs, cols

    p.s1, p.idx1, p.col1 = layer_streams(y0reg[es], y0idx[es])
    p.s2, p.idx2, p.col2 = layer_streams(y1reg[es], y1idx[es])
    p.prep_idx = [_wrap_idx16(prep_idx[k]) for k in range(c.NC)]
    p.dinv_p = [np.ascontiguousarray(dinv_p[k].reshape(PREP_TILES, 128).T) for k in range(c.NC)]
    p.dinv_d = [np.ascontiguousarray(dinv_d[k].reshape(c.NW, 128).T) for k in range(c.NC)]
    p.order_d = order_d
    p.lin2_npc = lin2(np.arange(c.NPC))
    return p


# ---------------- device kernel ----------------

def build_nc(p, sim_collectives=False, shared_tables=False, msg_bufs=3, mp_zpool=False, emb_bufs=2):
    c = p.cfg
    f32, bf16, i16 = mybir.dt.float32, mybir.dt.bfloat16, mybir.dt.int16
    D, DIN = c.D, c.DIN
    KIN = DIN // 128
    PC = c.PREP_CALL
    T1, T2 = p.s1["TOTBLK"], p.s2["TOTBLK"]
    N1, N2 = p.s1["NINST"], p.s2["NINST"]
    nc = bacc.Bacc("TRN2", target_bir_lowering=False, debug=False,
                   num_devices=c.NC, num_swdge_queues=c.NQ)

    embed = nc.dram_tensor("embed", [c.V, DIN], bf16, kind="ExternalInput").ap()
    prep_idx_d = nc.dram_tensor("prep_idx", [128, p.PREP_ROWS // 16], i16, kind="ExternalInput").ap()
    idx1_d = nc.dram_tensor("idx1", [128, T1 * 8], i16, kind="ExternalInput").ap()
    col1_d = nc.dram_tensor("col1", [128, N1], f32, kind="ExternalInput").ap()
    idx2_d = nc.dram_tensor("idx2", [128, T2 * 8], i16, kind="ExternalInput").ap()
    col2_d = nc.dram_tensor("col2", [128, N2], f32, kind="ExternalInput").ap()
    dinvp_d = nc.dram_tensor("dinvp", [128, p.PREP_TILES], f32, kind="ExternalInput").ap()
    dinvd_d = nc.dram_tensor("dinvd", [128, c.NW], f32, kind="ExternalInput").ap()
    a1t_d = nc.dram_tensor("a1t", [DIN, D], bf16, kind="ExternalInput").ap()
    w2t_d = nc.dram_tensor("w2t", [D, D], bf16, kind="ExternalInput").ap()
    iota_d = nc.dram_tensor("iota", [128, D], bf16, kind="ExternalInput").ap()
    outp = nc.dram_tensor("out", [c.DST_ROWS, D], f32, kind="ExternalOutput").ap()

    q1 = p.PREP_ROWS // c.NREG
    q2 = c.DST_ROWS // c.NREG
    CPP = p.NCALL1 // c.NREG          # prep calls per AllGather piece

    with tile.TileContext(nc) as tc:
        with (
            tc.tile_pool(name="dram", bufs=1, space="DRAM") as dpool,
            tc.tile_pool(name="const", bufs=1) as cpool,
            tc.tile_pool(name="msg", bufs=3) as msgpool,
            tc.tile_pool(name="emb", bufs=emb_bufs) as embpool,
            tc.tile_pool(name="s", bufs=8) as spool,
            tc.tile_pool(name="stg", bufs=2) as stgpool,
            tc.tile_pool(name="post", bufs=4) as postpool,
            tc.tile_pool(name="zp", bufs=c.SGW, space="PSUM") as zpool,
            tc.tile_pool(name="aux", bufs=1, space="PSUM") as auxpool,
        ):
            f_space = "Shared" if (shared_tables and not sim_collectives) else "Local"
            g1b = dpool.tile([p.PREP_ROWS, D], bf16)
            g1fs = [dpool.tile([p.REG1, D], bf16, addr_space=f_space,
                               name=f"g1f_r{r}") for r in range(c.NREG)]
            g2b = dpool.tile([c.DST_ROWS, D], bf16)
            g2fs = [dpool.tile([p.REG2, D], bf16, addr_space=f_space,
                               name=f"g2f_r{r}") for r in range(c.NREG)]

            iota_t = cpool.tile([128, D], bf16)
            nc.sync.dma_start(iota_t[:], iota_d[:])
            a1t_t = cpool.tile([128, KIN, D], bf16)
            for kk in range(KIN):
                nc.sync.dma_start(a1t_t[:, kk, :], a1t_d[kk * 128:(kk + 1) * 128, :])
            w2t_t = cpool.tile([128, D], bf16)
            nc.sync.dma_start(w2t_t[:], w2t_d[:])
            dinvp_t = cpool.tile([128, p.PREP_TILES], f32)
            nc.sync.dma_start(dinvp_t[:], dinvp_d[:])
            dinvd_t = cpool.tile([128, c.NW], f32)
            nc.sync.dma_start(dinvd_t[:], dinvd_d[:])
            dinvd2_t = cpool.tile([128, c.NW], f32)
            nc.vector.tensor_mul(dinvd2_t[:], dinvd_t[:], dinvd_t[:])
            pidx_t = cpool.tile([128, p.PREP_ROWS // 16], i16)
            nc.sync.dma_start(pidx_t[:], prep_idx_d[:])
            # bulk idx/col streams for both layers
            idx1_t = cpool.tile([128, T1 * 8], i16)
            nc.sync.dma_start(idx1_t[:], idx1_d[:])
            col1_t = cpool.tile([128, N1], f32)
            nc.sync.dma_start(col1_t[:], col1_d[:])
            idx2_t = cpool.tile([128, T2 * 8], i16)
            nc.sync.dma_start(idx2_t[:], idx2_d[:])
            col2_t = cpool.tile([128, N2], f32)
            nc.sync.dma_start(col2_t[:], col2_d[:])

            mcall = 0
            act_copy = mybir.ActivationFunctionType.Copy
            act_relu = mybir.ActivationFunctionType.Relu

            # ---------------- prep: g1 = dinv * (embed[tok] @ A1.T) ----------------
            halfA = embed[0:c.HALF, :]
            halfB = embed[c.HALF:c.V, :]
            n_callsA = p.NA // PC

            def emit_ag1(q):
                if sim_collectives:
                    for kk in range(c.NC):
                        nc.sync.dma_start(g1fs[q][kk * q1:(kk + 1) * q1, :],
                                          g1b[q * q1:(q + 1) * q1, :])
                    return
                nc.gpsimd.collective_compute(
                    "AllGather", mybir.AluOpType.bypass,
                    ins=[g1b[q * q1:(q + 1) * q1, :]],
                    outs=[g1fs[q][:]],
                    replica_groups=[list(range(c.NC))],
                )

            ag1_next = 0
            for call in range(p.NCALL1):
                while ag1_next < c.NREG and (ag1_next + 1) * CPP <= call:
                    emit_ag1(ag1_next)
                    ag1_next += 1
                srcap = halfA if call < n_callsA else halfB
                # transpose-mode gather: [din-part, KIN, tokens] bf16
                et = embpool.tile([128, KIN, PC], bf16, tag="emb")
                nc.gpsimd.dma_gather(
                    et[:], srcap, pidx_t[:, call * (PC // 16):(call + 1) * (PC // 16)],
                    num_idxs=PC, num_idxs_reg=PC, elem_size=DIN, transpose=True,
                    queue_num=0,
                )
                g1s = stgpool.tile([128, c.WB1, D], bf16, tag="g1s")
                for j in range(c.WB1):
                    t_idx = call * c.WB1 + j
                    if mp_zpool:
                        mpt = zpool.tile([128, D], f32, tag="z", name=f"mp_{t_idx}")
                    else:
                        mpt = auxpool.tile([128, D], f32, tag="aux", name=f"mp_{t_idx}")
                    for kk in range(KIN):
                        nc.tensor.matmul(mpt[:], et[:, kk, j * 128:(j + 1) * 128],
                                         a1t_t[:, kk, :],
                                         start=(kk == 0), stop=(kk == KIN - 1))
                    nc.scalar.activation(g1s[:, j, :], mpt[:], act_copy,
                                         scale=dinvp_t[:, t_idx:t_idx + 1])
                nc.sync.dma_start(g1b[call * PC:(call + 1) * PC, :], g1s[:])

            while ag1_next < c.NREG:
                emit_ag1(ag1_next)
                ag1_next += 1

            # ---------------- layers ----------------
            def layer(sched, idx_t, col_t, tables, is_last, grp_hook=None):
                groups, calls = sched["groups"], sched["calls"]
                call_insts = sched["call_insts"]
                first_inst, last_inst = sched["first_inst"], sched["last_inst"]
                nonlocal mcall
                call_i = 0
                ncalls = len(calls)
                zt = {}
                li = 2 if is_last else 1
                for gi, grp in enumerate(groups):
                    if grp_hook is not None:
                        grp_hook(gi)
                    for w in grp:
                        zt[w] = zpool.tile([128, D], f32, tag="z", name=f"zt{li}_w{w}")
                    while call_i < ncalls and calls[call_i][0] == gi:
                        _, r, b0, nbc, nreg = calls[call_i]
                        ci = call_insts[call_i]
                        call_i += 1
                        msg_t = msgpool.tile([128, c.CALLBLK, D], bf16, tag="msg")
                        nc.gpsimd.dma_gather(
                            msg_t[:, 0:nbc, :], tables[r][:],
                            idx_t[:, b0 * 8:(b0 + nbc) * 8],
                            num_idxs=nbc * 128, num_idxs_reg=nreg, elem_size=D,
                            queue_num=1 + mcall % 3,
                        )
                        mcall += 1
                        for iid, blk, w, kl in ci:
                            b = blk - b0
                            s_t = spool.tile([128, D], bf16, tag="s")
                            nc.vector.tensor_scalar(
                                out=s_t[:], in0=iota_t[:], scalar1=col_t[:, iid:iid + 1],
                                scalar2=None, op0=mybir.AluOpType.is_equal,
                            )
                            if is_last:   # zt[dst, feat] += S.T @ msg
                                nc.tensor.matmul(zt[w][:], s_t[0:kl, :], msg_t[0:kl, b, :],
                                                 start=(iid == first_inst[w]),
                                                 stop=(iid == last_inst[w]))
                            else:         # zt[feat, dst] += msg.T @ S
                                nc.tensor.matmul(zt[w][:], msg_t[0:kl, b, :], s_t[0:kl, :],
                                                 start=(iid == first_inst[w]),
                                                 stop=(iid == last_inst[w]))
                    # post-process this group's windows into a staged batch write
                    stage = stgpool.tile([128, c.SGW, D], f32 if is_last else bf16,
                                         tag="outs" if is_last else "g2s")
                    for jw, w in enumerate(grp):
                        if not is_last:
                            r1 = postpool.tile([128, D], bf16, tag="r1")
                            nc.scalar.activation(r1[:], zt[w][:], act_relu)
                            gp = auxpool.tile([128, D], f32, tag="aux")
                            nc.tensor.matmul(gp[:], r1[:], w2t_t[:], start=True, stop=True)
                            nc.scalar.activation(stage[:, jw, :], gp[:], act_copy,
                                                 scale=dinvd2_t[:, w:w + 1])
                        else:
                            nc.scalar.activation(stage[:, jw, :], zt[w][:], act_copy,
                                                 scale=dinvd_t[:, w:w + 1])
                        del zt[w]
                    dst = outp if is_last else g2b
                    nc.sync.dma_start(
                        dst[gi * c.SGW * 128:(gi + 1) * c.SGW * 128, :], stage[:])

            def emit_ag2(q):
                if sim_collectives:
                    for kk in range(c.NC):
                        nc.sync.dma_start(g2fs[q][kk * q2:(kk + 1) * q2, :],
                                          g2b[q * q2:(q + 1) * q2, :])
                    return
                nc.gpsimd.collective_compute(
                    "AllGather", mybir.AluOpType.bypass,
                    ins=[g2b[q * q2:(q + 1) * q2, :]],
                    outs=[g2fs[q][:]],
                    replica_groups=[list(range(c.NC))],
                )

            ag2_state = [0]

            def ag2_hook(gi):
                while (ag2_state[0] < c.NREG and
                       (ag2_state[0] + 1) * q2 <= gi * c.SGW * 128):
                    emit_ag2(ag2_state[0])
                    ag2_state[0] += 1

            layer(p.s1, idx1_t, col1_t, g1fs, is_last=False, grp_hook=ag2_hook)
            while ag2_state[0] < c.NREG:
                emit_ag2(ag2_state[0])
                ag2_state[0] += 1
            layer(p.s2, idx2_t, col2_t, g2fs, is_last=True)
    nc.finalize()
    return nc


# ---------------- host-side weight prep + in_maps ----------------

def make_in_maps(p, embed_table, W_node_w, W_node_b, conv1_w, conv1_b, conv2_w, conv2_b):
    c = p.cfg
    assert np.abs(W_node_b).max() == 0 and np.abs(conv1_b).max() == 0 and np.abs(conv2_b).max() == 0, \
        "nonzero biases not supported by this build (all-zero in this problem)"
    A1 = (np.asarray(conv1_w, np.float64) @ np.asarray(W_node_w, np.float64)).astype(np.float32)
    a1t = np.ascontiguousarray(A1.T).astype(BF)                  # [DIN, D]
    w2t = np.ascontiguousarray(np.asarray(conv2_w, np.float32).T).astype(BF)
    iota = np.tile(np.arange(c.D, dtype=np.float32), (128, 1)).astype(BF)
    emb = np.ascontiguousarray(np.asarray(embed_table, np.float32).astype(BF))
    maps = []
    for k in range(c.NC):
        maps.append({
            "embed": emb,
            "prep_idx": p.prep_idx[k],
            "idx1": p.idx1[k], "col1": p.col1[k],
            "idx2": p.idx2[k], "col2": p.col2[k],
            "dinvp": p.dinv_p[k], "dinvd": p.dinv_d[k],
            "a1t": a1t, "w2t": w2t, "iota": iota,
        })
    return maps


def assemble(p, results):
    c = p.cfg
    out = np.empty((c.N, c.D), np.float32)
    for k in range(c.NC):
        r = results[k]["out"]
        out[k * c.NPC + p.order_d[k]] = r[p.lin2_npc]
    return out


_CACHE = {}

def kernel(node_tokens, edge_index, embed_table, W_node_w, W_node_b,
           conv1_w, conv1_b, conv2_w, conv2_b):
    cfg = FULL
    p = preprocess(cfg, node_tokens, edge_index)
    key = (p.PREP_ROWS, p.s1["TOTBLK"], p.s2["TOTBLK"],
           tuple(p.s1["calls"]), tuple(p.s2["calls"]))
    if key not in _CACHE:
        _CACHE[key] = build_nc(p)
    nc = _CACHE[key]
    maps = make_in_maps(p, embed_table, W_node_w, W_node_b, conv1_w, conv1_b, conv2_w, conv2_b)
    res = run_bass_kernel_spmd(nc, maps, core_ids=list(range(cfg.NC)))
    return assemble(p, res.results)
